# revision 10
# baseline (speedup 1.0000x reference)
"""Trainium2 Bass kernel for nn_Enhancement_11819749999257.

Computes: 3x (1x1-conv MLP w/ BN+relu) feature embeddings + soft scatter of
per-joint features onto a 7x7 grid ("bone projection"), concatenated.

Full output: (256, 4736, 7, 7) f32 = 237 MB  -> memory(write)-bound.

Strategy (pure data parallel over batch, 8 cores x 32 batch items):
  - n = b_local*74 + j  flattens (batch item, joint). The per-core output
    (32, 4736, 49) is contiguous as rows n: out[n, c*49+s]. Rows are
    processed in 19 chunks of 128 partitions; each chunk's store is a
    [128 part x 12544 B contiguous] DMA (~1.57 MB) -> near peak HBM BW.
  - MLP: w1/w2 are 64x64; BN (eval) folded into per-channel scale/bias on
    host. PE matmuls: y1 = relu(scale*(w1 @ x) + bias) in 5 column pieces;
    per chunk F = [y1_chunk; ones].T @ [w2.T; b2] (K=65 folds the b2 add)
    lands in PSUM in [n-partition, c-free] layout (no transpose needed).
  - Grid weights W[n, s] = relu(1 - sqrt((gy_s+eps-u_n)^2 + (gx_s+eps-v_n)^2))
    computed in 5 batched pieces: ACT Square (per-partition bias = -uv),
    one DVE add per piece, ACT Sqrt, ACT Relu.
  - Scatter: OUT[n, c*49+s] = F[n, c] * W[n, s] -- one DVE tensor_tensor
    mult per chunk with stride-0 broadcast APs ([128,64,1] x [128,1,49]).
  - Inputs stream on the SWDGE queue; output stores alternate between the
    two HWDGE rings (sync / scalar).
"""

import numpy as np

import concourse.bass as bass
import concourse.mybir as mybir
from concourse import bacc, bass_utils
from concourse.tile import TileContext

F32 = mybir.dt.float32
AF = mybir.ActivationFunctionType
ALU = mybir.AluOpType

N_CORES = 8
B = 256
B_LOC = B // N_CORES      # 32
J = 74                    # 21 + 21 + 32 joints, concat order r, l, o
C = 64
S = 7
S2 = S * S                # 49
NLOC = B_LOC * J          # 2368 rows per core
P = 128
NCHUNK = (NLOC + P - 1) // P   # 19 (last chunk has 64 valid rows)
NPAD = NCHUNK * P         # 2432
OUT_COLS = C * S2         # 3136
EPS = 1.0e-6
NA = 512                  # phase-A column piece
NPIECE = (NPAD + NA - 1) // NA  # 5

# packed-constants column layout: [gyc|gxc|nuv|w1t|w2b|sc1|bi1]
OFF_GY = 0
OFF_GX = OFF_GY + S2            # 49
OFF_NUV = OFF_GX + S2           # 98
OFF_W1 = OFF_NUV + 2 * NCHUNK   # 136
OFF_W2B = OFF_W1 + C            # 200
OFF_SC = OFF_W2B + C            # 264
OFF_BI = OFF_SC + 1             # 265
NCONST = OFF_BI + 1             # 266


def _piece_chunks(a):
    return range(4 * a, min(4 * (a + 1), NCHUNK))


def _build_module():
    nc = bacc.Bacc(None)
    names = {}
    with TileContext(nc) as tc:
        with tc.tile_pool(name="dram", bufs=1, space="DRAM") as dram:
            xall = dram.tile((C, NPAD), F32, kind="ExternalInput", name="xall")
            cpk = dram.tile((P, NCONST), F32, kind="ExternalInput", name="cpk")
            out = dram.tile((NLOC, OUT_COLS), F32, kind="ExternalOutput", name="out")
            for key, ap in (("xall", xall), ("cpk", cpk), ("out", out)):
                names[key] = ap.tensor.name

            with (
                tc.tile_pool(name="consts", bufs=1) as cpool,
                tc.tile_pool(name="ps_a", bufs=2, space="PSUM") as ps_a,
                tc.tile_pool(name="ps_b", bufs=3, space="PSUM") as ps_b,
                tc.tile_pool(name="outs", bufs=8) as opool,
            ):
                # Warm the ACT LUTs (Square/Sqrt/Relu) on dummy data at t=0
                # so the ~1.3us table loads overlap the input DMA wait.
                scr = cpool.tile((1, 8), F32)
                scro = cpool.tile((1, 8), F32)
                nc.gpsimd.memset(scr[:], 0.0625)
                nc.scalar.activation(scro[:, 0:2], scr[:, 0:2], AF.Square)
                nc.scalar.activation(scro[:, 2:4], scr[:, 2:4], AF.Sqrt)
                nc.scalar.activation(scro[:, 4:6], scr[:, 4:6], AF.Relu)

                cpk_sb = cpool.tile((P, NCONST), F32)
                nc.sync.dma_start(out=cpk_sb[:], in_=cpk[:])
                gyc_sb = cpk_sb[:, OFF_GY : OFF_GY + S2]
                gxc_sb = cpk_sb[:, OFF_GX : OFF_GX + S2]
                nuv_sb = cpk_sb[:, OFF_NUV : OFF_NUV + 2 * NCHUNK]
                w1t_sb = cpk_sb[:C, OFF_W1 : OFF_W1 + C]
                w2b_sb = cpk_sb[: C + 1, OFF_W2B : OFF_W2B + C]
                sc1_sb = cpk_sb[:C, OFF_SC : OFF_SC + 1]
                bi1_sb = cpk_sb[:C, OFF_BI : OFF_BI + 1]

                x_sb = cpool.tile((C, NPAD), F32)
                y1e = cpool.tile((C + 1, NPAD), F32)
                nc.gpsimd.memset(y1e[C : C + 1, :], 1.0)

                # W pieces, batched: sq0/sq1/ss scratch, wv holds W[n, k*49+s]
                sq0 = cpool.tile((P, NCHUNK * S2), F32)
                sq1 = cpool.tile((P, NCHUNK * S2), F32)
                ss = cpool.tile((P, NCHUNK * S2), F32)
                wv = cpool.tile((P, NCHUNK * S2), F32)

                dma_out_engines = [nc.sync, nc.scalar]

                for a in range(NPIECE):
                    a0 = a * NA
                    aw = min(NA, NPAD - a0)
                    nc.scalar.dma_start(
                        out=x_sb[:, a0 : a0 + aw], in_=xall[:, a0 : a0 + aw]
                    )
                    ps1 = ps_a.tile((C, NA), F32, tag="ps1")
                    nc.tensor.matmul(
                        ps1[:, :aw], lhsT=w1t_sb, rhs=x_sb[:, a0 : a0 + aw]
                    )
                    nc.scalar.activation(
                        y1e[:C, a0 : a0 + aw], ps1[:, :aw], AF.Relu,
                        bias=bi1_sb, scale=sc1_sb,
                    )

                    # W + scatter for this piece's chunks. Piece 0 splits off
                    # chunk 0 alone so the first scatter starts ASAP.
                    pc = list(_piece_chunks(a))
                    groups = [pc[:1], pc[1:]] if a == 0 else [pc]
                    for grp in groups:
                        for k in grp:
                            nc.scalar.activation(
                                sq0[:, k * S2 : (k + 1) * S2], gyc_sb, AF.Square,
                                bias=nuv_sb[:, 2 * k : 2 * k + 1],
                            )
                            nc.scalar.activation(
                                sq1[:, k * S2 : (k + 1) * S2], gxc_sb, AF.Square,
                                bias=nuv_sb[:, 2 * k + 1 : 2 * k + 2],
                            )
                        psl = slice(grp[0] * S2, (grp[-1] + 1) * S2)
                        nc.vector.tensor_tensor(ss[:, psl], sq0[:, psl],
                                                sq1[:, psl], ALU.add)
                        nc.scalar.activation(sq0[:, psl], ss[:, psl], AF.Sqrt)
                        nc.scalar.activation(wv[:, psl], sq0[:, psl], AF.Relu,
                                             bias=1.0, scale=-1.0)

                        for k in grp:
                            rows = min(P, NLOC - k * P)
                            # F = [y1;1].T @ [w2t;b2] -> PSUM [128 (n), 64 (c)]
                            psf = ps_b.tile((P, C), F32, tag="psf")
                            nc.tensor.matmul(
                                psf[:], lhsT=y1e[:, k * P : (k + 1) * P],
                                rhs=w2b_sb,
                            )
                            # OUT[n, c*49+s] = F[n, c] * W[n, s]
                            o_sb = opool.tile((P, OUT_COLS), F32, tag="o")
                            f_bc, w_bc = bass.broadcast_tensor_aps(
                                psf[:, :, None],
                                wv[:, k * S2 : (k + 1) * S2][:, None, :],
                            )
                            o_3d = o_sb.rearrange("p (c s) -> p c s", s=S2)
                            nc.vector.tensor_tensor(o_3d, f_bc, w_bc, ALU.mult)
                            dma_out_engines[k % 2].dma_start(
                                out=out[k * P : k * P + rows, :],
                                in_=o_sb[:rows, :],
                            )
    nc.compile()
    return nc, names


_CACHE = {}


def _get_module():
    if "nc" not in _CACHE:
        _CACHE["nc"], _CACHE["names"] = _build_module()
    return _CACHE["nc"], _CACHE["names"]


def _prep_inputs(j2d_r, j2d_l, kp2d_o, feat_r, feat_l, feat_o,
                 w1, b1, bn_gamma, bn_beta, bn_mean, bn_var, w2, b2):
    """Host-side marshaling: shard batch, pack layouts. Returns in_maps."""
    f32 = np.float32
    # grid: grid[s] = (x[s%7], x[s//7]) with x = arange(7)+0.5
    x = (np.arange(S, dtype=f32) + 0.5)
    gy = np.tile(x, S) + EPS            # gy[s] = x[s%7] + eps
    gx = np.repeat(x, S) + EPS          # gx[s] = x[s//7] + eps
    gyc = np.broadcast_to(gy, (P, S2)).copy()
    gxc = np.broadcast_to(gx, (P, S2)).copy()

    scale = (bn_gamma.astype(f32) / np.sqrt(bn_var.astype(f32) + np.float32(1e-5)))
    bias1 = (b1.astype(f32) - bn_mean.astype(f32)) * scale + bn_beta.astype(f32)

    cpk0 = np.zeros((P, NCONST), f32)
    cpk0[:, OFF_GY : OFF_GY + S2] = gyc
    cpk0[:, OFF_GX : OFF_GX + S2] = gxc
    cpk0[:C, OFF_W1 : OFF_W1 + C] = w1.astype(f32).T
    cpk0[:C, OFF_W2B : OFF_W2B + C] = w2.astype(f32).T
    cpk0[C, OFF_W2B : OFF_W2B + C] = b2.astype(f32)
    cpk0[:C, OFF_SC] = scale
    cpk0[:C, OFF_BI] = bias1

    xcat = np.concatenate([feat_r, feat_l, feat_o], axis=2).astype(f32)  # (B,64,74)
    jcat = np.concatenate([j2d_r, j2d_l, kp2d_o], axis=1).astype(f32)   # (B,74,2)

    in_maps = []
    for c in range(N_CORES):
        sl = slice(c * B_LOC, (c + 1) * B_LOC)
        # xall[c_ch, n] = xcat[b', c_ch, j], n = b'*74+j ; pad n to 2432
        xc = np.transpose(xcat[sl], (1, 0, 2)).reshape(C, NLOC)
        xall = np.zeros((C, NPAD), f32)
        xall[:, :NLOC] = xc
        # nuv[p, 2k+i] = -(jcat[n=128k+p, i] + 1) * 3.5
        jc = np.zeros((NPAD, 2), f32)
        jc[:NLOC] = jcat[sl].reshape(NLOC, 2)
        nuv_flat = -(jc + np.float32(1.0)) * np.float32(3.5)
        cpk = cpk0.copy()
        cpk[:, OFF_NUV : OFF_NUV + 2 * NCHUNK] = (
            nuv_flat.reshape(NCHUNK, P, 2).transpose(1, 0, 2).reshape(P, 2 * NCHUNK)
        )
        in_maps.append(dict(xall=xall, cpk=cpk))
    return in_maps


def kernel_with_results(trace=False, **inputs):
    nc, names = _get_module()
    in_maps_l = _prep_inputs(**inputs)
    in_maps = [{names[k]: v for k, v in m.items()} for m in in_maps_l]
    res = bass_utils.run_bass_kernel_spmd(
        nc, in_maps, core_ids=list(range(N_CORES)), trace=trace
    )
    out_name = names["out"]
    parts = [
        res.results[c][out_name].reshape(B_LOC, J * C, S, S) for c in range(N_CORES)
    ]
    full = np.concatenate(parts, axis=0)
    return full, res


def kernel(**inputs):
    full, _ = kernel_with_results(trace=False, **inputs)
    return full


# revision 16
# speedup vs baseline: 1.1233x; 1.1233x over previous
"""Trainium2 Bass kernel for nn_Enhancement_11819749999257.

Computes: 3x (1x1-conv MLP w/ BN+relu) feature embeddings + soft scatter of
per-joint features onto a 7x7 grid ("bone projection"), concatenated.

Full output: (256, 4736, 7, 7) f32 = 237 MB  -> memory(write)-bound.

Strategy (pure data parallel over batch, 8 cores x 32 batch items):
  - n = b_local*74 + j  flattens (batch item, joint). The per-core output
    (32, 4736, 49) is contiguous as rows n: out[n, c*49+s]. Rows are
    processed in 19 chunks of 128 partitions; each chunk's store is a
    [128 part x 12544 B contiguous] DMA (~1.57 MB) -> near peak HBM BW.
  - MLP: w1/w2 are 64x64; BN (eval) folded into per-channel scale/bias on
    host. PE matmuls: y1 = relu(scale*(w1 @ x) + bias) in 5 column pieces;
    per chunk F = [y1_chunk; ones].T @ [w2.T; b2] (K=65 folds the b2 add)
    lands in PSUM in [n-partition, c-free] layout (no transpose needed).
  - Grid weights W[n, s] = relu(1 - sqrt((gy_s+eps-u_n)^2 + (gx_s+eps-v_n)^2))
    computed in 5 batched pieces: ACT Square (per-partition bias = -uv),
    one DVE add per piece, ACT Sqrt, ACT Relu.
  - Scatter: OUT[n, c*49+s] = F[n, c] * W[n, s] -- one DVE tensor_tensor
    mult per chunk with stride-0 broadcast APs ([128,64,1] x [128,1,49]).
  - Inputs stream on the SWDGE queue; output stores alternate between the
    two HWDGE rings (sync / scalar).
"""

import numpy as np

import concourse.bass as bass
import concourse.mybir as mybir
from concourse import bacc, bass_utils
from concourse.tile import TileContext

F32 = mybir.dt.float32
AF = mybir.ActivationFunctionType
ALU = mybir.AluOpType

N_CORES = 8
B = 256
B_LOC = B // N_CORES      # 32
J = 74                    # 21 + 21 + 32 joints, concat order r, l, o
C = 64
S = 7
S2 = S * S                # 49
NLOC = B_LOC * J          # 2368 rows per core
P = 128
NCHUNK = (NLOC + P - 1) // P   # 19 (last chunk has 64 valid rows)
NPAD = NCHUNK * P         # 2432
OUT_COLS = C * S2         # 3136
EPS = 1.0e-6
NA = 512                  # phase-A column piece
NPIECE = (NPAD + NA - 1) // NA  # 5

# packed-constants column layout, two tensors loaded on separate HWDGE rings:
# cpa = [w1t|w2b|sc1|bi1] (MLP path), cpb = [gyc|gxc|nuv] (grid path)
OFF_W1 = 0
OFF_W2B = OFF_W1 + C            # 64
OFF_SC = OFF_W2B + C            # 128
OFF_BI = OFF_SC + 1             # 129
NCONST_A = OFF_BI + 1           # 130
OFF_GY = 0
OFF_GX = OFF_GY + S2            # 49
OFF_NUV = OFF_GX + S2           # 98
NCONST_B = OFF_NUV + 2 * NCHUNK  # 136


def _piece_chunks(a):
    return range(4 * a, min(4 * (a + 1), NCHUNK))


def _build_module():
    nc = bacc.Bacc(None)
    names = {}
    with TileContext(nc) as tc:
        with tc.tile_pool(name="dram", bufs=1, space="DRAM") as dram:
            xall = dram.tile((C, NPAD), F32, kind="ExternalInput", name="xall")
            cpa = dram.tile((P, NCONST_A), F32, kind="ExternalInput", name="cpa")
            cpb = dram.tile((P, NCONST_B), F32, kind="ExternalInput", name="cpb")
            out = dram.tile((NLOC, OUT_COLS), F32, kind="ExternalOutput", name="out")
            for key, ap in (("xall", xall), ("cpa", cpa), ("cpb", cpb),
                            ("out", out)):
                names[key] = ap.tensor.name

            with (
                tc.tile_pool(name="consts", bufs=1) as cpool,
                tc.tile_pool(name="ps_a", bufs=2, space="PSUM") as ps_a,
                tc.tile_pool(name="ps_b", bufs=3, space="PSUM") as ps_b,
                tc.tile_pool(name="outs", bufs=8) as opool,
            ):
                # Warm the ACT LUTs (Square/Sqrt/Relu) on dummy data at t=0
                # so the ~1.3us table loads overlap the input DMA wait.
                scr = cpool.tile((1, 8), F32)
                scro = cpool.tile((1, 8), F32)
                nc.gpsimd.memset(scr[:], 0.0625)
                nc.scalar.activation(scro[:, 0:2], scr[:, 0:2], AF.Square)
                nc.scalar.activation(scro[:, 2:4], scr[:, 2:4], AF.Sqrt)
                nc.scalar.activation(scro[:, 4:6], scr[:, 4:6], AF.Relu)

                cpa_sb = cpool.tile((P, NCONST_A), F32)
                cpb_sb = cpool.tile((P, NCONST_B), F32)
                nc.sync.dma_start(out=cpa_sb[:], in_=cpa[:])
                nc.scalar.dma_start(out=cpb_sb[:], in_=cpb[:])
                gyc_sb = cpb_sb[:, OFF_GY : OFF_GY + S2]
                gxc_sb = cpb_sb[:, OFF_GX : OFF_GX + S2]
                nuv_sb = cpb_sb[:, OFF_NUV : OFF_NUV + 2 * NCHUNK]
                w1t_sb = cpa_sb[:C, OFF_W1 : OFF_W1 + C]
                w2b_sb = cpa_sb[: C + 1, OFF_W2B : OFF_W2B + C]
                sc1_sb = cpa_sb[:C, OFF_SC : OFF_SC + 1]
                bi1_sb = cpa_sb[:C, OFF_BI : OFF_BI + 1]

                x_sb = cpool.tile((C, NPAD), F32)
                y1e = cpool.tile((C + 1, NPAD), F32)
                nc.gpsimd.memset(y1e[C : C + 1, :], 1.0)

                # W pieces, batched: sq0/sq1/ss scratch, wv holds W[n, k*49+s]
                sq0 = cpool.tile((P, NCHUNK * S2), F32)
                sq1 = cpool.tile((P, NCHUNK * S2), F32)
                ss = cpool.tile((P, NCHUNK * S2), F32)
                wv = cpool.tile((P, NCHUNK * S2), F32)

                dma_out_engines = [nc.sync, nc.scalar]

                for a in range(NPIECE):
                    a0 = a * NA
                    aw = min(NA, NPAD - a0)
                    # piece 0 rides the sync ring (behind cpa only) so the
                    # first matmul can start as early as possible
                    (nc.sync if a == 0 else nc.scalar).dma_start(
                        out=x_sb[:, a0 : a0 + aw], in_=xall[:, a0 : a0 + aw]
                    )
                    ps1 = ps_a.tile((C, NA), F32, tag="ps1")
                    nc.tensor.matmul(
                        ps1[:, :aw], lhsT=w1t_sb, rhs=x_sb[:, a0 : a0 + aw]
                    )
                    nc.scalar.activation(
                        y1e[:C, a0 : a0 + aw], ps1[:, :aw], AF.Relu,
                        bias=bi1_sb, scale=sc1_sb,
                    )

                    # W + scatter for this piece's chunks. Piece 0 splits off
                    # chunk 0 alone so the first scatter starts ASAP.
                    pc = list(_piece_chunks(a))
                    groups = [pc[:1], pc[1:]] if a == 0 else [pc]
                    for grp in groups:
                        for k in grp:
                            nc.scalar.activation(
                                sq0[:, k * S2 : (k + 1) * S2], gyc_sb, AF.Square,
                                bias=nuv_sb[:, 2 * k : 2 * k + 1],
                            )
                            nc.scalar.activation(
                                sq1[:, k * S2 : (k + 1) * S2], gxc_sb, AF.Square,
                                bias=nuv_sb[:, 2 * k + 1 : 2 * k + 2],
                            )
                        psl = slice(grp[0] * S2, (grp[-1] + 1) * S2)
                        nc.vector.tensor_tensor(ss[:, psl], sq0[:, psl],
                                                sq1[:, psl], ALU.add)
                        nc.scalar.activation(sq0[:, psl], ss[:, psl], AF.Sqrt)
                        nc.scalar.activation(wv[:, psl], sq0[:, psl], AF.Relu,
                                             bias=1.0, scale=-1.0)

                        for k in grp:
                            rows = min(P, NLOC - k * P)
                            # F = [y1;1].T @ [w2t;b2] -> PSUM [128 (n), 64 (c)]
                            psf = ps_b.tile((P, C), F32, tag="psf")
                            nc.tensor.matmul(
                                psf[:], lhsT=y1e[:, k * P : (k + 1) * P],
                                rhs=w2b_sb,
                            )
                            # OUT[n, c*49+s] = F[n, c] * W[n, s]
                            o_sb = opool.tile((P, OUT_COLS), F32, tag="o")
                            f_bc, w_bc = bass.broadcast_tensor_aps(
                                psf[:, :, None],
                                wv[:, k * S2 : (k + 1) * S2][:, None, :],
                            )
                            o_3d = o_sb.rearrange("p (c s) -> p c s", s=S2)
                            nc.vector.tensor_tensor(o_3d, f_bc, w_bc, ALU.mult)
                            dma_out_engines[k % 2].dma_start(
                                out=out[k * P : k * P + rows, :],
                                in_=o_sb[:rows, :],
                            )
    nc.compile()
    return nc, names


_CACHE = {}


def _get_module():
    if "nc" not in _CACHE:
        _CACHE["nc"], _CACHE["names"] = _build_module()
    return _CACHE["nc"], _CACHE["names"]


def _prep_inputs(j2d_r, j2d_l, kp2d_o, feat_r, feat_l, feat_o,
                 w1, b1, bn_gamma, bn_beta, bn_mean, bn_var, w2, b2):
    """Host-side marshaling: shard batch, pack layouts. Returns in_maps."""
    f32 = np.float32
    # grid: grid[s] = (x[s%7], x[s//7]) with x = arange(7)+0.5
    x = (np.arange(S, dtype=f32) + 0.5)
    gy = np.tile(x, S) + EPS            # gy[s] = x[s%7] + eps
    gx = np.repeat(x, S) + EPS          # gx[s] = x[s//7] + eps
    gyc = np.broadcast_to(gy, (P, S2)).copy()
    gxc = np.broadcast_to(gx, (P, S2)).copy()

    scale = (bn_gamma.astype(f32) / np.sqrt(bn_var.astype(f32) + np.float32(1e-5)))
    bias1 = (b1.astype(f32) - bn_mean.astype(f32)) * scale + bn_beta.astype(f32)

    cpa0 = np.zeros((P, NCONST_A), f32)
    cpa0[:C, OFF_W1 : OFF_W1 + C] = w1.astype(f32).T
    cpa0[:C, OFF_W2B : OFF_W2B + C] = w2.astype(f32).T
    cpa0[C, OFF_W2B : OFF_W2B + C] = b2.astype(f32)
    cpa0[:C, OFF_SC] = scale
    cpa0[:C, OFF_BI] = bias1
    cpb0 = np.zeros((P, NCONST_B), f32)
    cpb0[:, OFF_GY : OFF_GY + S2] = gyc
    cpb0[:, OFF_GX : OFF_GX + S2] = gxc

    xcat = np.concatenate([feat_r, feat_l, feat_o], axis=2).astype(f32)  # (B,64,74)
    jcat = np.concatenate([j2d_r, j2d_l, kp2d_o], axis=1).astype(f32)   # (B,74,2)

    in_maps = []
    for c in range(N_CORES):
        sl = slice(c * B_LOC, (c + 1) * B_LOC)
        # xall[c_ch, n] = xcat[b', c_ch, j], n = b'*74+j ; pad n to 2432
        xc = np.transpose(xcat[sl], (1, 0, 2)).reshape(C, NLOC)
        xall = np.zeros((C, NPAD), f32)
        xall[:, :NLOC] = xc
        # nuv[p, 2k+i] = -(jcat[n=128k+p, i] + 1) * 3.5
        jc = np.zeros((NPAD, 2), f32)
        jc[:NLOC] = jcat[sl].reshape(NLOC, 2)
        nuv_flat = -(jc + np.float32(1.0)) * np.float32(3.5)
        cpb = cpb0.copy()
        cpb[:, OFF_NUV : OFF_NUV + 2 * NCHUNK] = (
            nuv_flat.reshape(NCHUNK, P, 2).transpose(1, 0, 2).reshape(P, 2 * NCHUNK)
        )
        in_maps.append(dict(xall=xall, cpa=cpa0, cpb=cpb))
    return in_maps


def kernel_with_results(trace=False, **inputs):
    nc, names = _get_module()
    in_maps_l = _prep_inputs(**inputs)
    in_maps = [{names[k]: v for k, v in m.items()} for m in in_maps_l]
    res = bass_utils.run_bass_kernel_spmd(
        nc, in_maps, core_ids=list(range(N_CORES)), trace=trace
    )
    out_name = names["out"]
    parts = [
        res.results[c][out_name].reshape(B_LOC, J * C, S, S) for c in range(N_CORES)
    ]
    full = np.concatenate(parts, axis=0)
    return full, res


def kernel(**inputs):
    full, _ = kernel_with_results(trace=False, **inputs)
    return full
